# revision 35
# baseline (speedup 1.0000x reference)
"""Energy Transformer descent kernel for 8 Trainium2 NeuronCores.

Problem: 12 steps of gradient descent on
  E(x) = -(1/beta) sum logsumexp(beta q k^T) - 0.5 sum relu(g xi^T)^2,
  g = LayerNorm(x; gamma, delta), q = g Wq_h, k = g Wk_h.

Sharding: data-parallel over batch B=4 -> core pairs (2b, 2b+1); within a
pair, core j takes attention heads j*6..j*6+5 and Hopfield memories
xi[j*1536:(j+1)*1536].  Both energy terms contribute additively to dE/dx
and LayerNorm-backward is linear in the upstream gradient, so each core
computes a partial dx and a pairwise AllReduce produces the full step.

Host-side preprocessing folds gamma and the attention scale into the
weights (delta must be zero, which the problem guarantees):
  Wq' = sqrt(beta) diag(gamma) Wq      (forward projections)
  WqT' = (1/sqrt(beta)) (diag(gamma) Wq)^T   (gradient projections)
  xi' = xi diag(gamma)

Scheduling notes (all confirmed against perfetto/NTFF traces):
  - xi/xiT live in SBUF for the whole kernel (no per-step HBM streaming).
  - every PE transpose runs in bf16 (f32r transposes lower to
    fp32_mode=HIGH, 4x slower); xhat itself is kept bf16-only.
  - attention heads are software-pipelined (head h-1's PT transposes and
    dqT/dkT matmuls execute while head h's exp/normalize runs on
    Scalar/Vector), with Hopfield-forward matmuls woven between heads so
    the in-order PE queue never drains (keeps the HAM clock gate at 8/8).
  - Scalar does only Exp inside the head loop (activation table stays
    resident); Pn normalization and relu drains run on Vector.
  - dgT accumulates in 6 PSUM banks (hopfield backward + attention
    backward); the first two chains start early, bridging head 5's
    softmax; then one bf16 transpose returns dg to [n,d] layout.
  - ~20 groups of dummy transposes gated on the AllReduce output re-warm
    the PE clock during the post-AR update/LN-stats window.  More is
    worse: they run in-order ahead of the real transposes.
  - fp8e4 DoubleRow for the Hopfield matmuls was tried and rejected:
    LDWEIGHTS-bound (no FWL in DoubleRow), and the e4m3 quantization of
    g/relu(h) accumulates to ~3.6e-2 relative error over 12 steps.
"""

import numpy as np

import concourse.bass as bass
import concourse.tile as tile
from concourse import bacc, mybir

STEPS = 12
ALPHA = 0.125
EPS = 1e-5
B, N, D, H, HD, M = 4, 512, 768, 12, 64, 3072
P = 128
NT = N // P  # 4 row chunks
DT = D // P  # 6 embed chunks
HL = H // 2  # heads per core
EW = HL * HD  # 384 local head width
ET = EW // P  # 3 stacked head-pair chunks
ML = M // 2  # memories per core
MT = ML // P  # 12 memory chunks
F32 = mybir.dt.float32
BF16 = mybir.dt.bfloat16
F8 = mybir.dt.float8e4
AF = mybir.ActivationFunctionType
OP = mybir.AluOpType
DR = mybir.MatmulPerfMode.DoubleRow

REPLICA_GROUPS = [[0, 1], [2, 3], [4, 5], [6, 7]]


def build_kernel(steps=STEPS, with_ar=True, debug_phase=99, debug_dump=False):
    nc = bacc.Bacc("TRN2", target_bir_lowering=False, debug=False, num_devices=8)

    x_in = nc.declare_dram_parameter("x", [N, D], F32, isOutput=False)
    wq_d = nc.declare_dram_parameter("wq", [D, EW], BF16, isOutput=False)
    wk_d = nc.declare_dram_parameter("wk", [D, EW], BF16, isOutput=False)
    wqt_d = nc.declare_dram_parameter("wqt", [EW, D], BF16, isOutput=False)
    wkt_d = nc.declare_dram_parameter("wkt", [EW, D], BF16, isOutput=False)
    xi_d = nc.declare_dram_parameter("xi", [ML, D], BF16, isOutput=False)
    xit_d = nc.declare_dram_parameter("xit", [D, ML], BF16, isOutput=False)
    x_out = nc.declare_dram_parameter("x_out", [N, D], F32, isOutput=True)
    dbg = {}
    if debug_dump:
        for nm, shp in (("xhat", [N, D]), ("gT", [D, N]), ("q", [N, EW]),
                        ("kT", [EW, N]), ("P0", [N, N]), ("dqT", [EW, N]),
                        ("dg", [N, D]), ("dx", [N, D])):
            dbg[nm] = nc.declare_dram_parameter("o_" + nm, shp, F32, isOutput=True)

    with tile.TileContext(nc) as tc:
        import contextlib

        with contextlib.ExitStack() as ctx:
            consts = ctx.enter_context(tc.tile_pool(name="consts", bufs=1))
            work = ctx.enter_context(tc.tile_pool(name="work", bufs=1))
            pp = ctx.enter_context(tc.tile_pool(name="pp", bufs=2))
            stats = ctx.enter_context(tc.tile_pool(name="stats", bufs=4))
            scr = ctx.enter_context(tc.tile_pool(name="scr", bufs=2))
            drp = ctx.enter_context(tc.tile_pool(name="drp", bufs=2, space="DRAM"))

            # ---- resident tensors ----
            wq_sb = consts.tile([P, DT, EW], BF16)
            nc.sync.dma_start(out=wq_sb[:], in_=wq_d.rearrange("(dt p) e -> p dt e", p=P))
            wk_sb = consts.tile([P, DT, EW], BF16)
            nc.sync.dma_start(out=wk_sb[:], in_=wk_d.rearrange("(dt p) e -> p dt e", p=P))
            wqt_sb = consts.tile([P, ET, D], BF16)
            nc.sync.dma_start(out=wqt_sb[:], in_=wqt_d.rearrange("(et p) d -> p et d", p=P))
            wkt_sb = consts.tile([P, ET, D], BF16)
            nc.sync.dma_start(out=wkt_sb[:], in_=wkt_d.rearrange("(et p) d -> p et d", p=P))
            xi_sb = consts.tile([P, MT, D], BF16)
            nc.sync.dma_start(out=xi_sb[:], in_=xi_d.rearrange("(mt p) d -> p mt d", p=P))
            xit_sb = consts.tile([P, DT, ML], BF16)
            nc.sync.dma_start(out=xit_sb[:], in_=xit_d.rearrange("(dt p) m -> p dt m", p=P))
            x_sb = consts.tile([P, NT, D], F32)
            nc.sync.dma_start(out=x_sb[:], in_=x_in.rearrange("(nt p) d -> p nt d", p=P))

            from concourse.masks import make_identity

            ident_f = consts.tile([P, P], F32)
            make_identity(nc, ident_f[:])
            ident_b = consts.tile([P, P], BF16)
            nc.vector.tensor_copy(out=ident_b[:], in_=ident_f[:])
            eps_t = consts.tile([P, 1], F32)
            nc.vector.memset(eps_t[:], EPS)

            for step in range(steps):
                # ======== A: LayerNorm forward + gT + q/k projections ========
                psA_ctx = tc.tile_pool(name="psA", bufs=2, space="PSUM")
                psA = psA_ctx.__enter__()
                psB_ctx = tc.tile_pool(name="psB", bufs=1, space="PSUM")
                psB = psB_ctx.__enter__()
                psC_ctx = tc.tile_pool(name="psC", bufs=2, space="PSUM")
                psC = psC_ctx.__enter__()
                xhb = work.tile([P, NT, D], BF16, tag="xhb")
                gT = work.tile([P, DT, N], BF16, tag="gT")
                q = work.tile([P, NT, EW], BF16, tag="q")
                k = work.tile([P, NT, EW], BF16, tag="k")
                qT = work.tile([P, ET, N], BF16, tag="qT")
                kT = work.tile([P, ET, N], BF16, tag="kT")
                rstd = stats.tile([P, NT], F32, tag="rstd")
                for nt in range(NT):
                    ns = slice(nt * P, (nt + 1) * P)
                    xt = x_sb[:, nt, :]
                    st = stats.tile([P, 3, 6], F32, tag="bnst")
                    xg = xt.rearrange("p (g s) -> p g s", s=256)
                    for gs in range(3):
                        nc.vector.bn_stats(out=st[:, gs, :], in_=xg[:, gs, :])
                    mv = stats.tile([P, 2], F32, tag="mv")
                    nc.vector.bn_aggr(out=mv[:], in_=st[:])
                    rr = rstd[:, nt : nt + 1]
                    nc.scalar.activation(out=rr, in_=mv[:, 1:2], func=AF.Sqrt, bias=eps_t[:], scale=1.0)
                    nc.vector.reciprocal(out=rr, in_=rr)
                    nmu = stats.tile([P, 1], F32, tag="nmu")
                    nc.vector.scalar_tensor_tensor(
                        out=nmu[:], in0=mv[:, 0:1], scalar=-1.0, in1=rr, op0=OP.mult, op1=OP.mult,
                    )
                    nc.scalar.activation(
                        out=xhb[:, nt, :], in_=xt, func=AF.Identity, scale=rr, bias=nmu[:],
                    )
                    gxp = psA.tile([P, DT, P], BF16, tag="gxp")
                    for dt in range(DT):
                        nc.tensor.transpose(gxp[:, dt, :], xhb[:, nt, dt * P : (dt + 1) * P], ident_b[:])
                    nc.vector.tensor_copy(out=gT[:, :, ns], in_=gxp[:])
                    ppq = psB.tile([P, 512], F32, tag="ppq")
                    ppk = psB.tile([P, 512], F32, tag="ppk")
                    for dt in range(DT):
                        lh = gT[:, dt, ns]
                        nc.tensor.matmul(ppq[:, :EW], lh, wq_sb[:, dt, :], start=(dt == 0), stop=(dt == DT - 1))
                    for dt in range(DT):
                        lh = gT[:, dt, ns]
                        nc.tensor.matmul(ppk[:, :EW], lh, wk_sb[:, dt, :], start=(dt == 0), stop=(dt == DT - 1))
                    nc.scalar.activation(out=q[:, nt, :], in_=ppq[:, :EW], func=AF.Identity)
                    nc.scalar.activation(out=k[:, nt, :], in_=ppk[:, :EW], func=AF.Identity)
                    qkxp = psC.tile([P, 2, ET, P], BF16, tag="qkxp")
                    for et in range(ET):
                        nc.tensor.transpose(qkxp[:, 0, et, :], q[:, nt, et * P : (et + 1) * P], ident_b[:])
                        nc.tensor.transpose(qkxp[:, 1, et, :], k[:, nt, et * P : (et + 1) * P], ident_b[:])
                    nc.vector.tensor_copy(out=qT[:, :, ns], in_=qkxp[:, 0, :, :])
                    nc.vector.tensor_copy(out=kT[:, :, ns], in_=qkxp[:, 1, :, :])

                psC_ctx.__exit__(None, None, None)
                psB_ctx.__exit__(None, None, None)
                psA_ctx.__exit__(None, None, None)

                if debug_dump and step == 0:
                    nc.sync.dma_start(out=dbg["xhat"].rearrange("(nt p) d -> p nt d", p=P), in_=xhb[:])
                    nc.sync.dma_start(out=dbg["gT"].rearrange("(dt p) n -> p dt n", p=P), in_=gT[:])
                    nc.sync.dma_start(out=dbg["q"].rearrange("(nt p) e -> p nt e", p=P), in_=q[:])
                    nc.sync.dma_start(out=dbg["kT"].rearrange("(et p) n -> p et n", p=P), in_=kT[:])
                if debug_phase < 3:
                    continue

                # ======== B1: attention heads with hopfield-forward woven in ====
                psD1_ctx = tc.tile_pool(name="psD1", bufs=1, space="PSUM")
                psD1 = psD1_ctx.__enter__()
                psS_ctx = tc.tile_pool(name="psS", bufs=3, space="PSUM")
                psS = psS_ctx.__enter__()
                psP_ctx = tc.tile_pool(name="psP", bufs=1, space="PSUM")
                psP = psP_ctx.__enter__()
                psH_ctx = tc.tile_pool(name="psH", bufs=2, space="PSUM")
                psH = psH_ctx.__enter__()

                RT = work.tile([P, MT, N], BF16, tag="RT")
                dqTst = work.tile([P, ET, N], BF16, tag="dqTst")
                dkTst = work.tile([P, ET, N], BF16, tag="dkTst")

                def hopf_fwd(mt):
                    hp = psH.tile([P, 512], F32, tag="hp")
                    for dt in range(DT):
                        nc.tensor.matmul(
                            hp[:], xit_sb[:, dt, mt * P : (mt + 1) * P], gT[:, dt, :],
                            start=(dt == 0), stop=(dt == DT - 1),
                        )
                    nc.vector.tensor_scalar_max(out=RT[:, mt, :], in0=hp[:], scalar1=0.0)

                def start_head(h):
                    """scores -> exp -> normalized Pn for head h."""
                    et, eo = h // 2, (h % 2) * HD
                    es = slice(eo, eo + HD)
                    Pn = pp.tile([P, NT, N], BF16, tag="Pn")
                    sm = stats.tile([P, NT], F32, tag="sm")
                    smr = stats.tile([P, NT], F32, tag="smr")
                    scs = []
                    for nt in range(NT):
                        sc = psS.tile([P, 512], F32, tag="ps")
                        nc.tensor.matmul(
                            sc[:], qT[es, et, nt * P : (nt + 1) * P], kT[es, et, :],
                            start=True, stop=True,
                        )
                        scs.append(sc)
                    for nt in range(NT):
                        nc.scalar.activation(
                            out=Pn[:, nt, :], in_=scs[nt][:], func=AF.Exp, bias=0.0, scale=1.0,
                            accum_out=sm[:, nt : nt + 1],
                        )
                    nc.vector.reciprocal(out=smr[:], in_=sm[:])
                    for nt in range(NT):
                        nc.vector.tensor_scalar_mul(
                            out=Pn[:, nt, :], in0=Pn[:, nt, :], scalar1=smr[:, nt : nt + 1]
                        )
                    return Pn

                def finish_head(h, Pn):
                    """PT transposes + dqT/dkT for head h (runs while head h+1's
                    softmax is on Scalar/Vector)."""
                    et, eo = h // 2, (h % 2) * HD
                    es = slice(eo, eo + HD)
                    hh = slice(h * HD, (h + 1) * HD)
                    PTn = pp.tile([P, NT, N], BF16, tag="PTn")
                    for mt2 in range(NT // 2):
                        pt = psP.tile([P, 2, 512], BF16, tag="pt")
                        for mi in range(2):
                            mt = 2 * mt2 + mi
                            for nt in range(NT):
                                nc.tensor.transpose(pt[:, mi, nt * P : (nt + 1) * P], Pn[:, nt, mt * P : (mt + 1) * P], ident_b[:])
                        nc.vector.tensor_copy(out=PTn[:, 2 * mt2 : 2 * mt2 + 2, :], in_=pt[:])
                    # dqT_h = sum_mt k_h[mt]^T-as-lhsT @ PT[mt]
                    dqp = psS.tile([P, 512], F32, tag="ps")
                    for mt in range(NT):
                        nc.tensor.matmul(
                            dqp[:HD, :], k[:, mt, hh], PTn[:, mt, :],
                            start=(mt == 0), stop=(mt == NT - 1),
                        )
                    nc.vector.tensor_copy(out=dqTst[es, et, :], in_=dqp[:HD, :])
                    # dkT_h = sum_nt q_h[nt]-as-lhsT @ P[nt]
                    dkp = psS.tile([P, 512], F32, tag="ps")
                    for nt in range(NT):
                        nc.tensor.matmul(
                            dkp[:HD, :], q[:, nt, hh], Pn[:, nt, :],
                            start=(nt == 0), stop=(nt == NT - 1),
                        )
                    nc.vector.tensor_copy(out=dkTst[es, et, :], in_=dkp[:HD, :])

                hopf_fwd(0)
                weave = {0: [1, 2], 1: [3, 4], 2: [5, 6], 3: [7, 8], 4: [9, 10], 5: [11]}
                prev = None
                for h in range(HL):
                    Pn_h = start_head(h)
                    mts = weave[h]
                    if mts:
                        hopf_fwd(mts[0])
                    if prev is not None:
                        finish_head(prev[0], prev[1])
                    for mt in mts[1:]:
                        hopf_fwd(mt)
                    if h == HL - 1:
                        # all hopfield-forward done: recycle psH banks to start
                        # the first two dgT chains while head 5's softmax runs
                        psH_ctx.__exit__(None, None, None)
                        dgTb01 = [psD1.tile([P, N], F32, tag=f"dgTa{dt}", name=f"dgTa{dt}") for dt in range(2)]
                        for dt in range(2):
                            ds = slice(dt * P, (dt + 1) * P)
                            for mt in range(MT):
                                nc.tensor.matmul(
                                    dgTb01[dt][:], xi_sb[:, mt, ds], RT[:, mt, :],
                                    start=(mt == 0), stop=False,
                                )
                    prev = (h, Pn_h)
                finish_head(prev[0], prev[1])

                psP_ctx.__exit__(None, None, None)
                psS_ctx.__exit__(None, None, None)

                if debug_dump and step == 0:
                    nc.sync.dma_start(out=dbg["dqT"].rearrange("(et p) n -> p et n", p=P), in_=dqTst[:])
                if debug_phase < 5:
                    continue

                # ======== B2: dgT accumulation (hopfield bwd + attention bwd) ===
                # dgT (= -true dg^T); each d-chunk owns a full PSUM bank.
                psD_ctx = tc.tile_pool(name="psD", bufs=1, space="PSUM")
                psD = psD_ctx.__enter__()
                dgTb = dgTb01 + [psD.tile([P, N], F32, tag=f"dgT{dt}", name=f"dgT{dt}") for dt in range(2, DT)]
                dgTs = work.tile([P, DT, N], BF16, tag="dgTs")
                for dt in range(2, DT):
                    ds = slice(dt * P, (dt + 1) * P)
                    for mt in range(MT):
                        nc.tensor.matmul(
                            dgTb[dt][:], xi_sb[:, mt, ds], RT[:, mt, :],
                            start=(mt == 0), stop=False,
                        )
                for dt in range(DT):
                    ds = slice(dt * P, (dt + 1) * P)
                    cnt = 0
                    for et in range(ET):
                        for d_t, w_t in ((dqTst, wqt_sb), (dkTst, wkt_sb)):
                            cnt += 1
                            nc.tensor.matmul(
                                dgTb[dt][:], w_t[:, et, ds], d_t[:, et, :],
                                start=False, stop=(cnt == 2 * ET),
                            )
                    eng = nc.vector if dt % 2 == 0 else nc.scalar
                    if dt % 2 == 0:
                        nc.vector.tensor_copy(out=dgTs[:, dt, :], in_=dgTb[dt][:])
                    else:
                        nc.scalar.activation(out=dgTs[:, dt, :], in_=dgTb[dt][:], func=AF.Identity)
                psD_ctx.__exit__(None, None, None)
                psD1_ctx.__exit__(None, None, None)

                # ======== tail: transpose dg back + LayerNorm backward ========
                psT_ctx = tc.tile_pool(name="psT", bufs=2, space="PSUM")
                psT = psT_ctx.__enter__()
                dxb = work.tile([P, NT, D], BF16, tag="dxb")
                for nt in range(NT):
                    ns = slice(nt * P, (nt + 1) * P)
                    rr = rstd[:, nt : nt + 1]
                    ptt1 = psT.tile([P, 4, P], BF16, tag="tt1")
                    ptt2 = psT.tile([P, 2, P], BF16, tag="tt2")
                    for dt in range(4):
                        nc.tensor.transpose(ptt1[:, dt, :], dgTs[:, dt, ns], ident_b[:])
                    for dt in range(4, DT):
                        nc.tensor.transpose(ptt2[:, dt - 4, :], dgTs[:, dt, ns], ident_b[:])
                    dy = scr.tile([P, D], F32, tag="dy")
                    m1a = stats.tile([P, 2], F32, tag="m1a")
                    nc.scalar.activation(
                        out=dy[:, 0:512], in_=ptt1[:], func=AF.Identity, accum_out=m1a[:, 0:1],
                    )
                    nc.scalar.activation(
                        out=dy[:, 512:768], in_=ptt2[:], func=AF.Identity, accum_out=m1a[:, 1:2],
                    )
                    m1 = stats.tile([P, 1], F32, tag="m1")
                    nc.vector.tensor_tensor(out=m1[:], in0=m1a[:, 0:1], in1=m1a[:, 1:2], op=OP.add)
                    prod = scr.tile([P, D], F32, tag="prod")
                    u2 = stats.tile([P, 1], F32, tag="u2")
                    nc.vector.scalar_tensor_tensor(
                        out=prod[:], in0=dy[:], scalar=1.0, in1=xhb[:, nt, :],
                        op0=OP.mult, op1=OP.mult, accum_out=u2[:],
                    )
                    c1n = stats.tile([P, 1], F32, tag="c1n")
                    nc.vector.scalar_tensor_tensor(
                        out=c1n[:], in0=m1[:], scalar=-1.0 / D, in1=rr, op0=OP.mult, op1=OP.mult,
                    )
                    c2 = stats.tile([P, 1], F32, tag="c2")
                    nc.vector.scalar_tensor_tensor(
                        out=c2[:], in0=u2[:], scalar=-1.0 / D, in1=rr, op0=OP.mult, op1=OP.mult,
                    )
                    lnv = scr.tile([P, D], F32, tag="lnv")
                    nc.scalar.activation(
                        out=lnv[:], in_=dy[:], func=AF.Identity, scale=rr, bias=c1n[:],
                    )
                    nc.vector.scalar_tensor_tensor(
                        out=dxb[:, nt, :], in0=xhb[:, nt, :], scalar=c2[:], in1=lnv[:],
                        op0=OP.mult, op1=OP.add,
                    )
                psT_ctx.__exit__(None, None, None)

                if debug_dump and step == 0:
                    nc.sync.dma_start(out=dbg["dx"].rearrange("(nt p) d -> p nt d", p=P), in_=dxb[:])

                # ======== pair AllReduce + update ========
                if with_ar:
                    arin = drp.tile([N, D], BF16, tag="arin")
                    arout = drp.tile([N, D], BF16, tag="arout")
                    for nt in range(NT):
                        nc.sync.dma_start(out=arin[nt * P : (nt + 1) * P, :], in_=dxb[:, nt, :])
                    nc.gpsimd.collective_compute(
                        "AllReduce", OP.add, replica_groups=REPLICA_GROUPS,
                        ins=[arin.opt()], outs=[arout.opt()],
                    )
                    axs = work.tile([P, NT, D], BF16, tag="axs")
                    for nt in range(NT):
                        nc.sync.dma_start(out=axs[:, nt, :], in_=arout[nt * P : (nt + 1) * P, :])
                    upd = axs
                else:
                    upd = dxb
                if debug_phase < 12:
                    continue
                # pre-warm the PE HAM clock gate during the update/LN-stats
                # window: ~5us of dummy transposes gated on the AllReduce
                # result, so the next step's matmul body starts at 2.4 GHz.
                if step + 1 < steps:
                    psW_ctx = tc.tile_pool(name="psW", bufs=2, space="PSUM")
                    psW = psW_ctx.__enter__()
                    for w2 in range(20):
                        dum = psW.tile([P, NT, P], BF16, tag="dum")
                        wc = (w2 % DT) * P
                        for nt in range(NT):
                            nc.tensor.transpose(
                                dum[:, nt, :], upd[:, 0, wc : wc + P], ident_b[:]
                            )
                    psW_ctx.__exit__(None, None, None)
                for nt in range(NT):
                    nc.vector.scalar_tensor_tensor(
                        out=x_sb[:, nt, :], in0=upd[:, nt, :], scalar=ALPHA, in1=x_sb[:, nt, :],
                        op0=OP.mult, op1=OP.add,
                    )

            for nt in range(NT):
                nc.sync.dma_start(out=x_out[nt * P : (nt + 1) * P, :], in_=x_sb[:, nt, :])

    nc.compile()
    return nc


def _prep_inputs(x, gamma, delta, Wq, Wk, xi):
    """Build the 8 per-core input dicts (host-side sharding + weight folding)."""
    assert np.allclose(delta, 0.0), "kernel requires delta == 0"
    beta_sqrt = np.float32(1.0 / np.sqrt(np.sqrt(np.float32(HD))))
    # sqrt(beta) = (1/sqrt(HD))^(1/2) = HD^(-1/4)
    g = gamma.astype(np.float32)
    in_maps = []
    for c in range(8):
        b, j = c // 2, c % 2
        hs = slice(j * HL, (j + 1) * HL)
        wq_l = (Wq[hs] * g[None, :, None]).transpose(1, 0, 2).reshape(D, EW)
        wk_l = (Wk[hs] * g[None, :, None]).transpose(1, 0, 2).reshape(D, EW)
        wqt_l = (Wq[hs] * g[None, :, None]).transpose(0, 2, 1).reshape(EW, D)
        wkt_l = (Wk[hs] * g[None, :, None]).transpose(0, 2, 1).reshape(EW, D)
        xi_l = xi[j * ML : (j + 1) * ML] * g[None, :]
        import ml_dtypes

        bf = ml_dtypes.bfloat16
        in_maps.append(
            {
                "x": np.ascontiguousarray(x[b]),
                "wq": np.ascontiguousarray(wq_l * beta_sqrt).astype(bf),
                "wk": np.ascontiguousarray(wk_l * beta_sqrt).astype(bf),
                "wqt": np.ascontiguousarray(wqt_l / beta_sqrt).astype(bf),
                "wkt": np.ascontiguousarray(wkt_l / beta_sqrt).astype(bf),
                "xi": np.ascontiguousarray(xi_l).astype(bf),
                "xit": np.ascontiguousarray(xi_l.T).astype(bf),
            }
        )
    return in_maps


_NC_CACHE = {}


def _get_nc(steps=STEPS, with_ar=True):
    key = (steps, with_ar)
    if key not in _NC_CACHE:
        _NC_CACHE[key] = build_kernel(steps, with_ar)
    return _NC_CACHE[key]


def kernel(x, gamma, delta, Wq, Wk, xi):
    from concourse.bass_utils import run_bass_kernel_spmd

    x = np.asarray(x, dtype=np.float32)
    in_maps = _prep_inputs(
        x,
        np.asarray(gamma, np.float32),
        np.asarray(delta, np.float32),
        np.asarray(Wq, np.float32),
        np.asarray(Wk, np.float32),
        np.asarray(xi, np.float32),
    )
    nc = _get_nc()
    res = run_bass_kernel_spmd(nc, in_maps, list(range(8)))
    out = np.stack([res.results[2 * b]["x_out"] for b in range(B)], axis=0)
    return out.astype(np.float32)


# revision 36
# speedup vs baseline: 1.0708x; 1.0708x over previous
"""Energy Transformer descent kernel for 8 Trainium2 NeuronCores.

Problem: 12 steps of gradient descent on
  E(x) = -(1/beta) sum logsumexp(beta q k^T) - 0.5 sum relu(g xi^T)^2,
  g = LayerNorm(x; gamma, delta), q = g Wq_h, k = g Wk_h.

Sharding: data-parallel over batch B=4 -> core pairs (2b, 2b+1); within a
pair, core j takes attention heads j*6..j*6+5 and Hopfield memories
xi[j*1536:(j+1)*1536].  Both energy terms contribute additively to dE/dx
and LayerNorm-backward is linear in the upstream gradient, so each core
computes a partial dx and a pairwise AllReduce produces the full step.

Host-side preprocessing folds gamma and the attention scale into the
weights (delta must be zero, which the problem guarantees):
  Wq' = sqrt(beta) diag(gamma) Wq      (forward projections)
  WqT' = (1/sqrt(beta)) (diag(gamma) Wq)^T   (gradient projections)
  xi' = xi diag(gamma)

Scheduling notes (all confirmed against perfetto/NTFF traces):
  - xi/xiT live in SBUF for the whole kernel (no per-step HBM streaming).
  - every PE transpose runs in bf16 (f32r transposes lower to
    fp32_mode=HIGH, 4x slower); xhat itself is kept bf16-only.
  - attention heads are software-pipelined (head h-1's PT transposes and
    dqT/dkT matmuls execute while head h's exp/normalize runs on
    Scalar/Vector), with Hopfield-forward matmuls woven between heads so
    the in-order PE queue never drains (keeps the HAM clock gate at 8/8).
  - Scalar does only Exp inside the head loop (activation table stays
    resident); Pn normalization and relu drains run on Vector.
  - dgT accumulates in 6 PSUM banks (hopfield backward + attention
    backward); the first two chains start early, bridging head 5's
    softmax; then one bf16 transpose returns dg to [n,d] layout.
  - ~20 groups of dummy transposes gated on the AllReduce output re-warm
    the PE clock during the post-AR update/LN-stats window.  More is
    worse: they run in-order ahead of the real transposes.
  - fp8e4 DoubleRow for the Hopfield matmuls was tried and rejected:
    LDWEIGHTS-bound (no FWL in DoubleRow), and the e4m3 quantization of
    g/relu(h) accumulates to ~3.6e-2 relative error over 12 steps.
"""

import numpy as np

import concourse.bass as bass
import concourse.tile as tile
from concourse import bacc, mybir

STEPS = 12
ALPHA = 0.125
EPS = 1e-5
B, N, D, H, HD, M = 4, 512, 768, 12, 64, 3072
P = 128
NT = N // P  # 4 row chunks
DT = D // P  # 6 embed chunks
HL = H // 2  # heads per core
EW = HL * HD  # 384 local head width
ET = EW // P  # 3 stacked head-pair chunks
ML = M // 2  # memories per core
MT = ML // P  # 12 memory chunks
F32 = mybir.dt.float32
BF16 = mybir.dt.bfloat16
F8 = mybir.dt.float8e4
AF = mybir.ActivationFunctionType
OP = mybir.AluOpType
DR = mybir.MatmulPerfMode.DoubleRow

REPLICA_GROUPS = [[0, 1], [2, 3], [4, 5], [6, 7]]


def build_kernel(steps=STEPS, with_ar=True, debug_phase=99, debug_dump=False):
    nc = bacc.Bacc("TRN2", target_bir_lowering=False, debug=False, num_devices=8)

    x_in = nc.declare_dram_parameter("x", [N, D], F32, isOutput=False)
    wq_d = nc.declare_dram_parameter("wq", [D, EW], BF16, isOutput=False)
    wk_d = nc.declare_dram_parameter("wk", [D, EW], BF16, isOutput=False)
    wqt_d = nc.declare_dram_parameter("wqt", [EW, D], BF16, isOutput=False)
    wkt_d = nc.declare_dram_parameter("wkt", [EW, D], BF16, isOutput=False)
    xi_d = nc.declare_dram_parameter("xi", [ML, D], BF16, isOutput=False)
    xit_d = nc.declare_dram_parameter("xit", [D, ML], BF16, isOutput=False)
    x_out = nc.declare_dram_parameter("x_out", [N, D], F32, isOutput=True)
    dbg = {}
    if debug_dump:
        for nm, shp in (("xhat", [N, D]), ("gT", [D, N]), ("q", [N, EW]),
                        ("kT", [EW, N]), ("P0", [N, N]), ("dqT", [EW, N]),
                        ("dg", [N, D]), ("dx", [N, D])):
            dbg[nm] = nc.declare_dram_parameter("o_" + nm, shp, F32, isOutput=True)

    with tile.TileContext(nc) as tc:
        import contextlib

        with contextlib.ExitStack() as ctx:
            consts = ctx.enter_context(tc.tile_pool(name="consts", bufs=1))
            work = ctx.enter_context(tc.tile_pool(name="work", bufs=1))
            pp = ctx.enter_context(tc.tile_pool(name="pp", bufs=2))
            stats = ctx.enter_context(tc.tile_pool(name="stats", bufs=4))
            scr = ctx.enter_context(tc.tile_pool(name="scr", bufs=2))
            drp = ctx.enter_context(tc.tile_pool(name="drp", bufs=2, space="DRAM"))

            # ---- resident tensors ----
            wq_sb = consts.tile([P, DT, EW], BF16)
            nc.sync.dma_start(out=wq_sb[:], in_=wq_d.rearrange("(dt p) e -> p dt e", p=P))
            wk_sb = consts.tile([P, DT, EW], BF16)
            nc.sync.dma_start(out=wk_sb[:], in_=wk_d.rearrange("(dt p) e -> p dt e", p=P))
            wqt_sb = consts.tile([P, ET, D], BF16)
            nc.sync.dma_start(out=wqt_sb[:], in_=wqt_d.rearrange("(et p) d -> p et d", p=P))
            wkt_sb = consts.tile([P, ET, D], BF16)
            nc.sync.dma_start(out=wkt_sb[:], in_=wkt_d.rearrange("(et p) d -> p et d", p=P))
            xi_sb = consts.tile([P, MT, D], BF16)
            nc.sync.dma_start(out=xi_sb[:], in_=xi_d.rearrange("(mt p) d -> p mt d", p=P))
            xit_sb = consts.tile([P, DT, ML], BF16)
            nc.sync.dma_start(out=xit_sb[:], in_=xit_d.rearrange("(dt p) m -> p dt m", p=P))
            x_sb = consts.tile([P, NT, D], F32)
            nc.sync.dma_start(out=x_sb[:], in_=x_in.rearrange("(nt p) d -> p nt d", p=P))

            from concourse.masks import make_identity

            ident_f = consts.tile([P, P], F32)
            make_identity(nc, ident_f[:])
            ident_b = consts.tile([P, P], BF16)
            nc.vector.tensor_copy(out=ident_b[:], in_=ident_f[:])
            eps_t = consts.tile([P, 1], F32)
            nc.vector.memset(eps_t[:], EPS)

            for step in range(steps):
                # ======== A: LayerNorm forward + gT + q/k projections ========
                psA_ctx = tc.tile_pool(name="psA", bufs=2, space="PSUM")
                psA = psA_ctx.__enter__()
                psB_ctx = tc.tile_pool(name="psB", bufs=1, space="PSUM")
                psB = psB_ctx.__enter__()
                psC_ctx = tc.tile_pool(name="psC", bufs=2, space="PSUM")
                psC = psC_ctx.__enter__()
                xhb = work.tile([P, NT, D], BF16, tag="xhb")
                gT = work.tile([P, DT, N], BF16, tag="gT")
                q = work.tile([P, NT, EW], BF16, tag="q")
                k = work.tile([P, NT, EW], BF16, tag="k")
                qT = work.tile([P, ET, N], BF16, tag="qT")
                kT = work.tile([P, ET, N], BF16, tag="kT")
                rstd = stats.tile([P, NT], F32, tag="rstd")
                for nt in range(NT):
                    ns = slice(nt * P, (nt + 1) * P)
                    xt = x_sb[:, nt, :]
                    st = stats.tile([P, 3, 6], F32, tag="bnst")
                    xg = xt.rearrange("p (g s) -> p g s", s=256)
                    for gs in range(3):
                        nc.vector.bn_stats(out=st[:, gs, :], in_=xg[:, gs, :])
                    mv = stats.tile([P, 2], F32, tag="mv")
                    nc.vector.bn_aggr(out=mv[:], in_=st[:])
                    rr = rstd[:, nt : nt + 1]
                    nc.scalar.activation(out=rr, in_=mv[:, 1:2], func=AF.Sqrt, bias=eps_t[:], scale=1.0)
                    nc.vector.reciprocal(out=rr, in_=rr)
                    nmu = stats.tile([P, 1], F32, tag="nmu")
                    nc.vector.scalar_tensor_tensor(
                        out=nmu[:], in0=mv[:, 0:1], scalar=-1.0, in1=rr, op0=OP.mult, op1=OP.mult,
                    )
                    nc.scalar.activation(
                        out=xhb[:, nt, :], in_=xt, func=AF.Identity, scale=rr, bias=nmu[:],
                    )
                    gxp = psA.tile([P, DT, P], BF16, tag="gxp")
                    for dt in range(DT):
                        nc.tensor.transpose(gxp[:, dt, :], xhb[:, nt, dt * P : (dt + 1) * P], ident_b[:])
                    nc.vector.tensor_copy(out=gT[:, :, ns], in_=gxp[:])
                    ppq = psB.tile([P, 512], F32, tag="ppq")
                    ppk = psB.tile([P, 512], F32, tag="ppk")
                    for dt in range(DT):
                        lh = gT[:, dt, ns]
                        nc.tensor.matmul(ppq[:, :EW], lh, wq_sb[:, dt, :], start=(dt == 0), stop=(dt == DT - 1))
                    for dt in range(DT):
                        lh = gT[:, dt, ns]
                        nc.tensor.matmul(ppk[:, :EW], lh, wk_sb[:, dt, :], start=(dt == 0), stop=(dt == DT - 1))
                    nc.scalar.activation(out=q[:, nt, :], in_=ppq[:, :EW], func=AF.Identity)
                    nc.scalar.activation(out=k[:, nt, :], in_=ppk[:, :EW], func=AF.Identity)
                    qkxp = psC.tile([P, 2, ET, P], BF16, tag="qkxp")
                    for et in range(ET):
                        nc.tensor.transpose(qkxp[:, 0, et, :], q[:, nt, et * P : (et + 1) * P], ident_b[:])
                        nc.tensor.transpose(qkxp[:, 1, et, :], k[:, nt, et * P : (et + 1) * P], ident_b[:])
                    nc.vector.tensor_copy(out=qT[:, :, ns], in_=qkxp[:, 0, :, :])
                    nc.vector.tensor_copy(out=kT[:, :, ns], in_=qkxp[:, 1, :, :])

                psC_ctx.__exit__(None, None, None)
                psB_ctx.__exit__(None, None, None)
                psA_ctx.__exit__(None, None, None)

                if debug_dump and step == 0:
                    nc.sync.dma_start(out=dbg["xhat"].rearrange("(nt p) d -> p nt d", p=P), in_=xhb[:])
                    nc.sync.dma_start(out=dbg["gT"].rearrange("(dt p) n -> p dt n", p=P), in_=gT[:])
                    nc.sync.dma_start(out=dbg["q"].rearrange("(nt p) e -> p nt e", p=P), in_=q[:])
                    nc.sync.dma_start(out=dbg["kT"].rearrange("(et p) n -> p et n", p=P), in_=kT[:])
                if debug_phase < 3:
                    continue

                # ======== B1: attention heads with hopfield-forward woven in ====
                psD1_ctx = tc.tile_pool(name="psD1", bufs=1, space="PSUM")
                psD1 = psD1_ctx.__enter__()
                psS_ctx = tc.tile_pool(name="psS", bufs=3, space="PSUM")
                psS = psS_ctx.__enter__()
                psP_ctx = tc.tile_pool(name="psP", bufs=1, space="PSUM")
                psP = psP_ctx.__enter__()
                psH_ctx = tc.tile_pool(name="psH", bufs=2, space="PSUM")
                psH = psH_ctx.__enter__()

                RT = work.tile([P, MT, N], BF16, tag="RT")
                dqTst = work.tile([P, ET, N], BF16, tag="dqTst")
                dkTst = work.tile([P, ET, N], BF16, tag="dkTst")

                def hopf_fwd(mt):
                    hp = psH.tile([P, 512], F32, tag="hp")
                    for dt in range(DT):
                        nc.tensor.matmul(
                            hp[:], xit_sb[:, dt, mt * P : (mt + 1) * P], gT[:, dt, :],
                            start=(dt == 0), stop=(dt == DT - 1),
                        )
                    nc.vector.tensor_scalar_max(out=RT[:, mt, :], in0=hp[:], scalar1=0.0)

                def start_head(h):
                    """scores -> exp -> normalized Pn for head h."""
                    et, eo = h // 2, (h % 2) * HD
                    es = slice(eo, eo + HD)
                    Pn = pp.tile([P, NT, N], BF16, tag="Pn")
                    sm = stats.tile([P, NT], F32, tag="sm")
                    smr = stats.tile([P, NT], F32, tag="smr")
                    scs = []
                    for nt in range(NT):
                        sc = psS.tile([P, 512], F32, tag="ps")
                        nc.tensor.matmul(
                            sc[:], qT[es, et, nt * P : (nt + 1) * P], kT[es, et, :],
                            start=True, stop=True,
                        )
                        scs.append(sc)
                    for nt in range(NT):
                        nc.scalar.activation(
                            out=Pn[:, nt, :], in_=scs[nt][:], func=AF.Exp, bias=0.0, scale=1.0,
                            accum_out=sm[:, nt : nt + 1],
                        )
                    nc.vector.reciprocal(out=smr[:], in_=sm[:])
                    for nt in range(NT):
                        nc.vector.tensor_scalar_mul(
                            out=Pn[:, nt, :], in0=Pn[:, nt, :], scalar1=smr[:, nt : nt + 1]
                        )
                    return Pn

                def finish_head(h, Pn):
                    """PT transposes + dqT/dkT for head h (runs while head h+1's
                    softmax is on Scalar/Vector)."""
                    et, eo = h // 2, (h % 2) * HD
                    es = slice(eo, eo + HD)
                    hh = slice(h * HD, (h + 1) * HD)
                    PTn = pp.tile([P, NT, N], BF16, tag="PTn")
                    for mt2 in range(NT // 2):
                        pt = psP.tile([P, 2, 512], BF16, tag="pt")
                        for mi in range(2):
                            mt = 2 * mt2 + mi
                            for nt in range(NT):
                                nc.tensor.transpose(pt[:, mi, nt * P : (nt + 1) * P], Pn[:, nt, mt * P : (mt + 1) * P], ident_b[:])
                        nc.vector.tensor_copy(out=PTn[:, 2 * mt2 : 2 * mt2 + 2, :], in_=pt[:])
                    # dqT_h = sum_mt k_h[mt]^T-as-lhsT @ PT[mt]
                    dqp = psS.tile([P, 512], F32, tag="ps")
                    for mt in range(NT):
                        nc.tensor.matmul(
                            dqp[:HD, :], k[:, mt, hh], PTn[:, mt, :],
                            start=(mt == 0), stop=(mt == NT - 1),
                        )
                    nc.vector.tensor_copy(out=dqTst[es, et, :], in_=dqp[:HD, :])
                    # dkT_h = sum_nt q_h[nt]-as-lhsT @ P[nt]
                    dkp = psS.tile([P, 512], F32, tag="ps")
                    for nt in range(NT):
                        nc.tensor.matmul(
                            dkp[:HD, :], q[:, nt, hh], Pn[:, nt, :],
                            start=(nt == 0), stop=(nt == NT - 1),
                        )
                    nc.vector.tensor_copy(out=dkTst[es, et, :], in_=dkp[:HD, :])

                hopf_fwd(0)
                weave = {0: [1, 2], 1: [3, 4], 2: [5, 6], 3: [7, 8], 4: [9, 10], 5: [11]}
                prev = None
                for h in range(HL):
                    Pn_h = start_head(h)
                    mts = weave[h]
                    if mts:
                        hopf_fwd(mts[0])
                    if prev is not None:
                        finish_head(prev[0], prev[1])
                    for mt in mts[1:]:
                        hopf_fwd(mt)
                    if h == HL - 1:
                        # all hopfield-forward done: recycle psH banks to start
                        # the first two dgT chains while head 5's softmax runs
                        psH_ctx.__exit__(None, None, None)
                        dgTb01 = [psD1.tile([P, N], F32, tag=f"dgTa{dt}", name=f"dgTa{dt}") for dt in range(2)]
                        for dt in range(2):
                            ds = slice(dt * P, (dt + 1) * P)
                            for mt in range(MT):
                                nc.tensor.matmul(
                                    dgTb01[dt][:], xi_sb[:, mt, ds], RT[:, mt, :],
                                    start=(mt == 0), stop=False,
                                )
                    prev = (h, Pn_h)
                finish_head(prev[0], prev[1])

                psP_ctx.__exit__(None, None, None)
                psS_ctx.__exit__(None, None, None)

                if debug_dump and step == 0:
                    nc.sync.dma_start(out=dbg["dqT"].rearrange("(et p) n -> p et n", p=P), in_=dqTst[:])
                if debug_phase < 5:
                    continue

                # ======== B2: dgT accumulation (hopfield bwd + attention bwd) ===
                # dgT (= -true dg^T); each d-chunk owns a full PSUM bank.
                psD_ctx = tc.tile_pool(name="psD", bufs=1, space="PSUM")
                psD = psD_ctx.__enter__()
                dgTb = dgTb01 + [psD.tile([P, N], F32, tag=f"dgT{dt}", name=f"dgT{dt}") for dt in range(2, DT)]
                dgTs = work.tile([P, DT, N], BF16, tag="dgTs")
                for dt in range(2, DT):
                    ds = slice(dt * P, (dt + 1) * P)
                    for mt in range(MT):
                        nc.tensor.matmul(
                            dgTb[dt][:], xi_sb[:, mt, ds], RT[:, mt, :],
                            start=(mt == 0), stop=False,
                        )
                for dt in range(DT):
                    ds = slice(dt * P, (dt + 1) * P)
                    cnt = 0
                    for et in range(ET):
                        for d_t, w_t in ((dqTst, wqt_sb), (dkTst, wkt_sb)):
                            cnt += 1
                            nc.tensor.matmul(
                                dgTb[dt][:], w_t[:, et, ds], d_t[:, et, :],
                                start=False, stop=(cnt == 2 * ET),
                            )
                    eng = nc.vector if dt % 2 == 0 else nc.scalar
                    if dt % 2 == 0:
                        nc.vector.tensor_copy(out=dgTs[:, dt, :], in_=dgTb[dt][:])
                    else:
                        nc.scalar.activation(out=dgTs[:, dt, :], in_=dgTb[dt][:], func=AF.Identity)
                psD_ctx.__exit__(None, None, None)
                psD1_ctx.__exit__(None, None, None)

                # ======== tail: transpose dg back + LayerNorm backward ========
                psT_ctx = tc.tile_pool(name="psT", bufs=2, space="PSUM")
                psT = psT_ctx.__enter__()
                dxb = work.tile([P, NT, D], BF16, tag="dxb")
                for nt in range(NT):
                    ns = slice(nt * P, (nt + 1) * P)
                    rr = rstd[:, nt : nt + 1]
                    ptt = psT.tile([P, DT, P], BF16, tag="tt")
                    for dt in range(DT):
                        nc.tensor.transpose(ptt[:, dt, :], dgTs[:, dt, ns], ident_b[:])
                    dy = scr.tile([P, D], F32, tag="dy")
                    m1 = stats.tile([P, 1], F32, tag="m1")
                    nc.scalar.activation(
                        out=dy[:], in_=ptt[:], func=AF.Identity, accum_out=m1[:],
                    )
                    prod = scr.tile([P, D], F32, tag="prod")
                    u2 = stats.tile([P, 1], F32, tag="u2")
                    nc.vector.scalar_tensor_tensor(
                        out=prod[:], in0=dy[:], scalar=1.0, in1=xhb[:, nt, :],
                        op0=OP.mult, op1=OP.mult, accum_out=u2[:],
                    )
                    c1n = stats.tile([P, 1], F32, tag="c1n")
                    nc.vector.scalar_tensor_tensor(
                        out=c1n[:], in0=m1[:], scalar=-1.0 / D, in1=rr, op0=OP.mult, op1=OP.mult,
                    )
                    c2 = stats.tile([P, 1], F32, tag="c2")
                    nc.vector.scalar_tensor_tensor(
                        out=c2[:], in0=u2[:], scalar=-1.0 / D, in1=rr, op0=OP.mult, op1=OP.mult,
                    )
                    lnv = scr.tile([P, D], F32, tag="lnv")
                    nc.scalar.activation(
                        out=lnv[:], in_=dy[:], func=AF.Identity, scale=rr, bias=c1n[:],
                    )
                    nc.vector.scalar_tensor_tensor(
                        out=dxb[:, nt, :], in0=xhb[:, nt, :], scalar=c2[:], in1=lnv[:],
                        op0=OP.mult, op1=OP.add,
                    )
                psT_ctx.__exit__(None, None, None)

                if debug_dump and step == 0:
                    nc.sync.dma_start(out=dbg["dx"].rearrange("(nt p) d -> p nt d", p=P), in_=dxb[:])

                # ======== pair AllReduce + update ========
                if with_ar:
                    arin = drp.tile([N, D], BF16, tag="arin")
                    arout = drp.tile([N, D], BF16, tag="arout")
                    for nt in range(NT):
                        nc.sync.dma_start(out=arin[nt * P : (nt + 1) * P, :], in_=dxb[:, nt, :])
                    nc.gpsimd.collective_compute(
                        "AllReduce", OP.add, replica_groups=REPLICA_GROUPS,
                        ins=[arin.opt()], outs=[arout.opt()],
                    )
                    axs = work.tile([P, NT, D], BF16, tag="axs")
                    for nt in range(NT):
                        nc.sync.dma_start(out=axs[:, nt, :], in_=arout[nt * P : (nt + 1) * P, :])
                    upd = axs
                else:
                    upd = dxb
                if debug_phase < 12:
                    continue
                # pre-warm the PE HAM clock gate during the update/LN-stats
                # window: ~5us of dummy transposes gated on the AllReduce
                # result, so the next step's matmul body starts at 2.4 GHz.
                if step + 1 < steps:
                    psW_ctx = tc.tile_pool(name="psW", bufs=2, space="PSUM")
                    psW = psW_ctx.__enter__()
                    for w2 in range(20):
                        dum = psW.tile([P, NT, P], BF16, tag="dum")
                        wc = (w2 % DT) * P
                        for nt in range(NT):
                            nc.tensor.transpose(
                                dum[:, nt, :], upd[:, 0, wc : wc + P], ident_b[:]
                            )
                    psW_ctx.__exit__(None, None, None)
                for nt in range(NT):
                    nc.vector.scalar_tensor_tensor(
                        out=x_sb[:, nt, :], in0=upd[:, nt, :], scalar=ALPHA, in1=x_sb[:, nt, :],
                        op0=OP.mult, op1=OP.add,
                    )

            for nt in range(NT):
                nc.sync.dma_start(out=x_out[nt * P : (nt + 1) * P, :], in_=x_sb[:, nt, :])

    nc.compile()
    return nc


def _prep_inputs(x, gamma, delta, Wq, Wk, xi):
    """Build the 8 per-core input dicts (host-side sharding + weight folding)."""
    assert np.allclose(delta, 0.0), "kernel requires delta == 0"
    beta_sqrt = np.float32(1.0 / np.sqrt(np.sqrt(np.float32(HD))))
    # sqrt(beta) = (1/sqrt(HD))^(1/2) = HD^(-1/4)
    g = gamma.astype(np.float32)
    in_maps = []
    for c in range(8):
        b, j = c // 2, c % 2
        hs = slice(j * HL, (j + 1) * HL)
        wq_l = (Wq[hs] * g[None, :, None]).transpose(1, 0, 2).reshape(D, EW)
        wk_l = (Wk[hs] * g[None, :, None]).transpose(1, 0, 2).reshape(D, EW)
        wqt_l = (Wq[hs] * g[None, :, None]).transpose(0, 2, 1).reshape(EW, D)
        wkt_l = (Wk[hs] * g[None, :, None]).transpose(0, 2, 1).reshape(EW, D)
        xi_l = xi[j * ML : (j + 1) * ML] * g[None, :]
        import ml_dtypes

        bf = ml_dtypes.bfloat16
        in_maps.append(
            {
                "x": np.ascontiguousarray(x[b]),
                "wq": np.ascontiguousarray(wq_l * beta_sqrt).astype(bf),
                "wk": np.ascontiguousarray(wk_l * beta_sqrt).astype(bf),
                "wqt": np.ascontiguousarray(wqt_l / beta_sqrt).astype(bf),
                "wkt": np.ascontiguousarray(wkt_l / beta_sqrt).astype(bf),
                "xi": np.ascontiguousarray(xi_l).astype(bf),
                "xit": np.ascontiguousarray(xi_l.T).astype(bf),
            }
        )
    return in_maps


_NC_CACHE = {}


def _get_nc(steps=STEPS, with_ar=True):
    key = (steps, with_ar)
    if key not in _NC_CACHE:
        _NC_CACHE[key] = build_kernel(steps, with_ar)
    return _NC_CACHE[key]


def kernel(x, gamma, delta, Wq, Wk, xi):
    from concourse.bass_utils import run_bass_kernel_spmd

    x = np.asarray(x, dtype=np.float32)
    in_maps = _prep_inputs(
        x,
        np.asarray(gamma, np.float32),
        np.asarray(delta, np.float32),
        np.asarray(Wq, np.float32),
        np.asarray(Wk, np.float32),
        np.asarray(xi, np.float32),
    )
    nc = _get_nc()
    res = run_bass_kernel_spmd(nc, in_maps, list(range(8)))
    out = np.stack([res.results[2 * b]["x_out"] for b in range(B)], axis=0)
    return out.astype(np.float32)
